# revision 6
# baseline (speedup 1.0000x reference)
"""AMMLinear (VQ codebook) forward on 8 TRN2 NeuronCores.

The straight-through estimator makes the forward VALUE exactly
    out[n, o] = sum_c lut_dq[c, argmin_k dist(x_cn, cent_ck), o] + bias[o]
with lut = centroids @ weight (per codebook) and lut_dq a global-scale int8
quantize-dequantize of lut.  The softmax/attention path only shapes gradients.

Sharding: tokens (BN=4096 -> 512/core) for the score/argmin phase, output
features (4096 -> 512/core) for the lut/gather phase.  One AllGather moves the
bf16 one-hot codes (1MB/core), one AllReduce-max the quantization scale.
Output is assembled host-side by concatenating the per-core o-shards.
"""

import numpy as np

import concourse.bass as bass
import concourse.mybir as mybir
import concourse.tile as tile
import concourse.bass_isa as bass_isa
from concourse import bacc
from concourse.bass_utils import run_bass_kernel_spmd
from concourse.masks import make_identity

F32 = mybir.dt.float32
BF16 = mybir.dt.bfloat16

N_CORES = 8
NC, K, IN_F, OUT_F = 64, 16, 4096, 4096
SUBV = IN_F // NC          # 64
BN = 4096                  # 2*2048 tokens
TOK = BN // N_CORES        # 512 tokens per core
NT = TOK // 128            # 4 token tiles per core
NPAIR = NC // 2            # 32 codebook pairs
CK = NC * K                # 1024 (codebook,centroid) flat index
NCHUNK = CK // 128         # 8 contraction chunks
OSH = OUT_F // N_CORES     # 512 out features per core
MAGIC = 12582912.0         # 1.5 * 2**23: fp32 round-to-nearest-even trick
BIG = 1024.0

_CACHE = {}


def _build():
    nc = bacc.Bacc("TRN2", target_bir_lowering=False, debug=False,
                   num_devices=N_CORES)

    xT = nc.declare_dram_parameter("xT", [IN_F, TOK], F32, isOutput=False)
    cbd = nc.declare_dram_parameter("cbd", [128, CK], F32, isOutput=False)
    c2r = nc.declare_dram_parameter("c2r", [128, CK], F32, isOutput=False)
    iotar = nc.declare_dram_parameter("iotar", [128, CK], F32, isOutput=False)
    wsh = nc.declare_dram_parameter("wsh", [IN_F, OSH], F32, isOutput=False)
    biasr = nc.declare_dram_parameter("biasr", [128, OSH], F32, isOutput=False)
    out = nc.declare_dram_parameter("out", [BN, OSH], F32, isOutput=True)

    with tile.TileContext(nc) as tc:
        with (
            tc.tile_pool(name="consts", bufs=1) as constp,
            tc.tile_pool(name="xt", bufs=4) as xtp,
            tc.tile_pool(name="wt", bufs=4) as wp,
            tc.tile_pool(name="work", bufs=2) as workp,
            tc.tile_pool(name="sone", bufs=4) as sp,
            tc.tile_pool(name="stl", bufs=1) as stlp,
            tc.tile_pool(name="stg", bufs=16) as stgp,
            tc.tile_pool(name="lut", bufs=1) as lutp,
            tc.tile_pool(name="outs", bufs=4) as outp,
            tc.tile_pool(name="ps", bufs=8, space="PSUM") as psp,
            tc.tile_pool(name="dram", bufs=1, space="DRAM") as dramp,
        ):
            # ---- constants -------------------------------------------------
            cbd_sb = constp.tile([128, CK], F32, tag="cbd_sb")
            nc.sync.dma_start(out=cbd_sb[:, :], in_=cbd.ap()[:, :])
            c2_sb = constp.tile([128, CK], F32, tag="c2_sb")
            nc.sync.dma_start(out=c2_sb[:, :], in_=c2r.ap()[:, :])
            iota_sb = constp.tile([128, CK], F32, tag="iota_sb")
            nc.sync.dma_start(out=iota_sb[:, :], in_=iotar.ap()[:, :])
            bias_sb = constp.tile([128, OSH], F32, tag="bias_sb")
            nc.sync.dma_start(out=bias_sb[:, :], in_=biasr.ap()[:, :])
            ident = constp.tile([128, 128], BF16, tag="ident")
            make_identity(nc, ident[:, :])

            # DRAM scratch for collectives
            st_bounce = dramp.tile([CK, TOK], BF16, tag="st_bounce")
            st_all = dramp.tile([N_CORES * CK, TOK], BF16, tag="st_all")
            mx_in = dramp.tile([1, 16], F32, tag="mx_in")
            mx_out = dramp.tile([1, 16], F32, tag="mx_out")

            # ---- phase 1: scores + argmin + one-hot ------------------------
            # score[n, ck] = c2[ck] - 2 * x.cent; xc via PE (x^T stationary,
            # 2-codebook block-diag centroids moving), fp32 exact.
            # stl_sb[:, TOK*j + n] holds S^T[128j + p, n] (chunk-major cols).
            stl_sb = stlp.tile([128, NCHUNK * TOK], BF16, tag="stl")
            s_tiles = [workp.tile([128, CK], F32, tag=f"score{t}", name=f"score{t}")
                       for t in range(NT)]
            for h in range(2):
                ps_sc = [psp.tile([128, 512], F32, tag="ps", name=f"ps_sc{h}{t}")
                         for t in range(NT)]
                for p16 in range(16):
                    p = 16 * h + p16
                    xt_t = xtp.tile([128, TOK], F32, tag="xt")
                    nc.sync.dma_start(out=xt_t[:, :],
                                      in_=xT.ap()[128 * p:128 * (p + 1), :])
                    for t in range(NT):
                        nc.tensor.matmul(
                            ps_sc[t][:, 32 * p16:32 * (p16 + 1)],
                            lhsT=xt_t[:, 128 * t:128 * (t + 1)],
                            rhs=cbd_sb[:, 32 * p:32 * (p + 1)],
                            start=True, stop=True,
                        )
                for t in range(NT):
                    nc.vector.scalar_tensor_tensor(
                        out=s_tiles[t][:, 512 * h:512 * (h + 1)],
                        in0=ps_sc[t][:, :],
                        scalar=-2.0,
                        in1=c2_sb[:, 512 * h:512 * (h + 1)],
                        op0=mybir.AluOpType.mult,
                        op1=mybir.AluOpType.add,
                    )

            for t in range(NT):
                score = s_tiles[t]
                score3 = score[:, :].rearrange("p (c k) -> p c k", k=K)
                m = workp.tile([128, NC], F32, tag="m")
                nc.vector.tensor_reduce(
                    out=m[:, :], in_=score3, axis=mybir.AxisListType.X,
                    op=mybir.AluOpType.min,
                )
                eq = workp.tile([128, CK], F32, tag="eq")
                nc.vector.tensor_tensor(
                    out=eq[:, :].rearrange("p (c k) -> p c k", k=K),
                    in0=score3,
                    in1=m[:, :].unsqueeze(2).broadcast_to([128, NC, K]),
                    op=mybir.AluOpType.is_equal,
                )
                cand = workp.tile([128, CK], F32, tag="cand")
                nc.vector.scalar_tensor_tensor(
                    out=cand[:, :], in0=eq[:, :], scalar=-BIG,
                    in1=iota_sb[:, :],
                    op0=mybir.AluOpType.mult, op1=mybir.AluOpType.add,
                )
                kmin = workp.tile([128, NC], F32, tag="kmin")
                nc.vector.tensor_reduce(
                    out=kmin[:, :],
                    in_=cand[:, :].rearrange("p (c k) -> p c k", k=K),
                    axis=mybir.AxisListType.X, op=mybir.AluOpType.min,
                )
                nc.vector.tensor_scalar_add(kmin[:, :], kmin[:, :], BIG)
                s_one = sp.tile([128, CK], BF16, tag="sone")
                nc.vector.tensor_tensor(
                    out=s_one[:, :].rearrange("p (c k) -> p c k", k=K),
                    in0=iota_sb[:, :].rearrange("p (c k) -> p c k", k=K),
                    in1=kmin[:, :].unsqueeze(2).broadcast_to([128, NC, K]),
                    op=mybir.AluOpType.is_equal,
                )
                # transpose one-hot [n, ck] -> [ck, n] per 128-chunk
                for j in range(NCHUNK):
                    ps_tr = psp.tile([128, 128], BF16, tag="ps")
                    nc.tensor.transpose(
                        ps_tr[:, :], s_one[:, 128 * j:128 * (j + 1)],
                        ident[:, :],
                    )
                    nc.vector.tensor_copy(
                        out=stl_sb[:, TOK * j + 128 * t:TOK * j + 128 * (t + 1)],
                        in_=ps_tr[:, :],
                    )

            # ---- phase 2: AllGather the one-hot codes ----------------------
            for j in range(NCHUNK):
                nc.sync.dma_start(
                    out=st_bounce[128 * j:128 * (j + 1), :],
                    in_=stl_sb[:, TOK * j:TOK * (j + 1)],
                )
            nc.gpsimd.collective_compute(
                "AllGather",
                mybir.AluOpType.bypass,
                replica_groups=[list(range(N_CORES))],
                ins=[st_bounce.opt()],
                outs=[st_all.opt()],
            )

            # ---- phase 3: lut = centroids @ weight (fp32, 4-way col pack) --
            lut_sb = lutp.tile([128, NCHUNK * OSH], F32, tag="lut")
            for j in range(NCHUNK):
                ps_lut = psp.tile([128, OSH], F32, tag="ps")
                for mcol in range(4):
                    p = 4 * j + mcol
                    w_t = wp.tile([128, OSH], F32, tag="wt")
                    nc.sync.dma_start(
                        out=w_t[:, :],
                        in_=wsh.ap()[128 * p:128 * (p + 1), :])
                    nc.tensor.matmul(
                        ps_lut[32 * mcol:32 * (mcol + 1), :],
                        lhsT=cbd_sb[:, 32 * p:32 * (p + 1)],
                        rhs=w_t[:, :],
                        start=True, stop=True,
                        tile_position=(0, 32 * mcol),
                    )
                nc.vector.tensor_copy(
                    out=lut_sb[:, OSH * j:OSH * (j + 1)], in_=ps_lut[:, :])

            # ---- phase 4: global absmax -> AllReduce(max) -> scale ---------
            mx8 = constp.tile([128, NCHUNK], F32, tag="mx8")
            for j in range(NCHUNK):
                nc.vector.tensor_reduce(
                    out=mx8[:, j:j + 1],
                    in_=lut_sb[:, OSH * j:OSH * (j + 1)],
                    axis=mybir.AxisListType.X, op=mybir.AluOpType.max,
                    apply_absolute_value=True,
                )
            mxl = constp.tile([128, 1], F32, tag="mxl")
            nc.vector.tensor_reduce(
                out=mxl[:, :], in_=mx8[:, :], axis=mybir.AxisListType.X,
                op=mybir.AluOpType.max,
            )
            mxp = constp.tile([128, 1], F32, tag="mxp")
            nc.gpsimd.partition_all_reduce(
                mxp[:, :], mxl[:, :], channels=128,
                reduce_op=bass_isa.ReduceOp.max,
            )
            zrow = constp.tile([1, 16], F32, tag="zrow")
            nc.vector.memset(zrow[:, :], 0.0)
            nc.vector.tensor_copy(out=zrow[0:1, 0:1], in_=mxp[0:1, 0:1])
            nc.sync.dma_start(out=mx_in[:, :], in_=zrow[:, :])
            nc.gpsimd.collective_compute(
                "AllReduce",
                mybir.AluOpType.max,
                replica_groups=[list(range(N_CORES))],
                ins=[mx_in.opt()],
                outs=[mx_out.opt()],
            )
            gmax_row = constp.tile([1, 16], F32, tag="gmax_row")
            nc.sync.dma_start(out=gmax_row[:, :], in_=mx_out[:, :])
            gmax = constp.tile([128, 1], F32, tag="gmax")
            nc.gpsimd.partition_broadcast(gmax[:, :], gmax_row[0:1, 0:1])
            # s = gmax/127 and 1/s via reciprocal (DVE tensor_scalar has no
            # divide); the <=2ulp drift moves ~1e-5 of entries across a
            # rounding boundary -- well inside the error budget.
            s_col = constp.tile([128, 1], F32, tag="s_col")
            nc.vector.tensor_scalar(
                out=s_col[:, :], in0=gmax[:, :], scalar1=float(np.float32(1.0) / np.float32(127.0)),
                scalar2=None, op0=mybir.AluOpType.mult,
            )
            rgmax = constp.tile([128, 1], F32, tag="rgmax")
            nc.vector.reciprocal(rgmax[:, :], gmax[:, :])
            inv_s = constp.tile([128, 1], F32, tag="inv_s")
            nc.vector.tensor_scalar(
                out=inv_s[:, :], in0=rgmax[:, :], scalar1=127.0, scalar2=None,
                op0=mybir.AluOpType.mult,
            )

            # ---- phase 5: quantize-dequantize lut -> bf16 ------------------
            lutdq = lutp.tile([128, NCHUNK * OSH], BF16, tag="lutdq")
            for j in range(NCHUNK):
                qm = wp.tile([128, OSH], F32, tag="qm")
                nc.vector.tensor_scalar(
                    out=qm[:, :], in0=lut_sb[:, OSH * j:OSH * (j + 1)],
                    scalar1=inv_s[:, 0:1], scalar2=MAGIC,
                    op0=mybir.AluOpType.mult, op1=mybir.AluOpType.add,
                )
                nc.vector.tensor_scalar(
                    out=lutdq[:, OSH * j:OSH * (j + 1)], in0=qm[:, :],
                    scalar1=MAGIC, scalar2=s_col[:, 0:1],
                    op0=mybir.AluOpType.subtract, op1=mybir.AluOpType.mult,
                )

            # ---- phase 6: gather matmul out = S @ lut_dq + bias ------------
            for r in range(N_CORES):
                stg_tiles = []
                for j in range(NCHUNK):
                    stg_t = stgp.tile([128, TOK], BF16, tag="stg")
                    nc.sync.dma_start(
                        out=stg_t[:, :],
                        in_=st_all[CK * r + 128 * j:CK * r + 128 * (j + 1), :],
                    )
                    stg_tiles.append(stg_t)
                for t in range(NT):
                    ps_o = psp.tile([128, OSH], F32, tag="ps")
                    for j in range(NCHUNK):
                        nc.tensor.matmul(
                            ps_o[:, :],
                            lhsT=stg_tiles[j][:, 128 * t:128 * (t + 1)],
                            rhs=lutdq[:, OSH * j:OSH * (j + 1)],
                            start=(j == 0), stop=(j == NCHUNK - 1),
                        )
                    o_sb = outp.tile([128, OSH], F32, tag="outsb")
                    nc.vector.tensor_tensor(
                        out=o_sb[:, :], in0=ps_o[:, :], in1=bias_sb[:, :],
                        op=mybir.AluOpType.add,
                    )
                    row0 = TOK * r + 128 * t
                    nc.sync.dma_start(
                        out=out.ap()[row0:row0 + 128, :], in_=o_sb[:, :])

    nc.compile()
    return nc


def _prep_inputs(x, centroids, weight, bias):
    x = np.ascontiguousarray(np.asarray(x, dtype=np.float32)).reshape(BN, IN_F)
    cent = np.asarray(centroids, dtype=np.float32)
    w = np.asarray(weight, dtype=np.float32)
    bias = np.asarray(bias, dtype=np.float32)

    c2 = (cent ** 2).sum(axis=-1).reshape(CK)  # [1024] flat (c,k)
    c2r = np.ascontiguousarray(np.broadcast_to(c2, (128, CK)))
    iota = np.tile(np.arange(K, dtype=np.float32), NC)
    iotar = np.ascontiguousarray(np.broadcast_to(iota, (128, CK)))
    cbd = np.zeros((128, CK), np.float32)
    for p in range(NPAIR):
        cbd[0:SUBV, 32 * p:32 * p + K] = cent[2 * p].T
        cbd[SUBV:128, 32 * p + K:32 * p + 2 * K] = cent[2 * p + 1].T

    in_maps = []
    for r in range(N_CORES):
        xT_r = np.ascontiguousarray(x[TOK * r:TOK * (r + 1)].T)
        w_r = np.ascontiguousarray(w[:, :, OSH * r:OSH * (r + 1)]).reshape(
            IN_F, OSH)
        bias_r = np.ascontiguousarray(
            np.broadcast_to(bias[OSH * r:OSH * (r + 1)], (128, OSH)))
        in_maps.append({
            "xT": xT_r, "cbd": cbd, "c2r": c2r, "iotar": iotar,
            "wsh": w_r, "biasr": bias_r,
        })
    return in_maps


def kernel(x, centroids, weight, inverse_temperature_logit, bias,
           **_unused) -> np.ndarray:
    if "nc" not in _CACHE:
        _CACHE["nc"] = _build()
    nc = _CACHE["nc"]
    in_maps = _prep_inputs(x, centroids, weight, bias)
    res = run_bass_kernel_spmd(nc, in_maps, core_ids=list(range(N_CORES)))
    out = np.concatenate([res.results[r]["out"] for r in range(N_CORES)],
                         axis=1)
    return out.reshape(2, BN // 2, OUT_F).astype(np.float32)


# revision 8
# speedup vs baseline: 1.3731x; 1.3731x over previous
"""AMMLinear (VQ codebook) forward on 8 TRN2 NeuronCores.

The straight-through estimator makes the forward VALUE exactly
    out[n, o] = sum_c lut_dq[c, argmin_k dist(x_cn, cent_ck), o] + bias[o]
with lut = centroids @ weight (per codebook) and lut_dq a global-scale int8
quantize-dequantize of lut.  The softmax/attention path only shapes gradients.

Sharding: tokens (BN=4096 -> 512/core) for the score/argmin phase, output
features (4096 -> 512/core) for the lut/gather phase.  One AllGather moves the
bf16 argmin indices (64KB/core), one AllReduce-max the quantization scale.
Every core then expands all 4096 tokens' one-hot codes locally (replication
matmul + is_equal) and computes its o-shard of the gather matmul.
Output is assembled host-side by concatenating the per-core o-shards.
"""

import numpy as np

import concourse.bass as bass
import concourse.mybir as mybir
import concourse.tile as tile
import concourse.bass_isa as bass_isa
from concourse import bacc
from concourse.bass_utils import run_bass_kernel_spmd
from concourse.masks import make_identity

F32 = mybir.dt.float32
F32R = mybir.dt.float32r
BF16 = mybir.dt.bfloat16

N_CORES = 8
NC, K, IN_F, OUT_F = 64, 16, 4096, 4096
SUBV = IN_F // NC          # 64
BN = 4096                  # 2*2048 tokens
TOK = BN // N_CORES        # 512 tokens per core
NT = TOK // 128            # 4 token tiles per core
NPAIR = NC // 2            # 32 codebook pairs
CK = NC * K                # 1024 (codebook,centroid) flat index
NCHUNK = CK // 128         # 8 contraction chunks
OSH = OUT_F // N_CORES     # 512 out features per core
MAGIC = 12582912.0         # 1.5 * 2**23: fp32 round-to-nearest-even trick
BIG = 1024.0

_CACHE = {}


def _build():
    nc = bacc.Bacc("TRN2", target_bir_lowering=False, debug=False,
                   num_devices=N_CORES)

    xT = nc.declare_dram_parameter("xT", [IN_F, TOK], F32, isOutput=False)
    cbd = nc.declare_dram_parameter("cbd", [128, CK], F32, isOutput=False)
    c2r = nc.declare_dram_parameter("c2r", [128, CK], F32, isOutput=False)
    iotar = nc.declare_dram_parameter("iotar", [128, CK], F32, isOutput=False)
    wsh_h = nc.declare_dram_parameter("wsh_h", [IN_F, OSH], BF16, isOutput=False)
    wsh_l = nc.declare_dram_parameter("wsh_l", [IN_F, OSH], BF16, isOutput=False)
    cbd_h = nc.declare_dram_parameter("cbd_h", [128, CK], BF16, isOutput=False)
    cbd_l = nc.declare_dram_parameter("cbd_l", [128, CK], BF16, isOutput=False)
    biasr = nc.declare_dram_parameter("biasr", [128, OSH], F32, isOutput=False)
    emat = nc.declare_dram_parameter("emat", [NC, CK], BF16, isOutput=False)
    kcol = nc.declare_dram_parameter("kcol", [128, 1], F32, isOutput=False)
    out = nc.declare_dram_parameter("out", [BN, OSH], BF16, isOutput=True)

    with tile.TileContext(nc) as tc:
        with (
            tc.tile_pool(name="consts", bufs=1) as constp,
            tc.tile_pool(name="xt", bufs=4) as xtp,
            tc.tile_pool(name="wt", bufs=4) as wp,
            tc.tile_pool(name="xct", bufs=3) as xctp,
            tc.tile_pool(name="work", bufs=2) as workp,
            tc.tile_pool(name="stg", bufs=16) as stgp,
            tc.tile_pool(name="lut", bufs=1) as lutp,
            tc.tile_pool(name="outs", bufs=4) as outp,
            tc.tile_pool(name="ps", bufs=8, space="PSUM") as psp,
            tc.tile_pool(name="dram", bufs=1, space="DRAM") as dramp,
        ):
            # ---- constants -------------------------------------------------
            cbd_sb = constp.tile([128, CK], F32, tag="cbd_sb")
            nc.sync.dma_start(out=cbd_sb[:, :], in_=cbd.ap()[:, :])
            c2_sb = constp.tile([128, CK], F32, tag="c2_sb")
            nc.sync.dma_start(out=c2_sb[:, :], in_=c2r.ap()[:, :])
            iota_sb = constp.tile([128, CK], F32, tag="iota_sb")
            nc.sync.dma_start(out=iota_sb[:, :], in_=iotar.ap()[:, :])
            bias_sb = constp.tile([128, OSH], F32, tag="bias_sb")
            nc.sync.dma_start(out=bias_sb[:, :], in_=biasr.ap()[:, :])
            emat_sb = constp.tile([NC, CK], BF16, tag="emat_sb")
            nc.sync.dma_start(out=emat_sb[:, :], in_=emat.ap()[:, :])
            kcol_sb = constp.tile([128, 1], F32, tag="kcol_sb")
            nc.sync.dma_start(out=kcol_sb[:, :], in_=kcol.ap()[:, :])
            identb = constp.tile([128, 128], BF16, tag="identb")
            make_identity(nc, identb[:, :])
            identf = constp.tile([128, 128], F32, tag="identf")
            make_identity(nc, identf[:, :])
            # bf16 hi/lo centroid blockdiag for the 3-pass lut matmul
            cbdh_sb = constp.tile([128, CK], BF16, tag="cbdh_sb")
            nc.sync.dma_start(out=cbdh_sb[:, :], in_=cbd_h.ap()[:, :])
            cbdl_sb = constp.tile([128, CK], BF16, tag="cbdl_sb")
            nc.sync.dma_start(out=cbdl_sb[:, :], in_=cbd_l.ap()[:, :])

            # DRAM scratch for collectives
            kt_bounce = dramp.tile([NC, TOK], BF16, tag="kt_bounce")
            kt_all = dramp.tile([N_CORES * NC, TOK], BF16, tag="kt_all")
            mx_in = dramp.tile([1, 16], F32, tag="mx_in")
            mx_out = dramp.tile([1, 16], F32, tag="mx_out")

            # ---- phase 1: scores (transposed), argmin, kmin^T --------------
            # xc^T[ck, n] via cent-stationary fp32 matmuls (4 pairs col-packed
            # per 128-row chunk), PE-transposed back to score[n, ck].
            s_tiles = [workp.tile([128, CK], F32, tag=f"score{t}",
                                  name=f"score{t}") for t in range(NT)]
            for j in range(NCHUNK):
                ps_xct = psp.tile([128, TOK], F32, tag="ps")
                for mcol in range(4):
                    p = 4 * j + mcol
                    xt_t = xtp.tile([128, TOK], F32, tag="xt")
                    nc.sync.dma_start(out=xt_t[:, :],
                                      in_=xT.ap()[128 * p:128 * (p + 1), :])
                    nc.tensor.matmul(
                        ps_xct[32 * mcol:32 * (mcol + 1), :],
                        lhsT=cbd_sb[:, 32 * p:32 * (p + 1)],
                        rhs=xt_t[:, :],
                        start=True, stop=True,
                        tile_position=(0, 32 * mcol),
                    )
                xct_sb = xctp.tile([128, TOK], F32, tag="xct")
                nc.vector.tensor_copy(out=xct_sb[:, :], in_=ps_xct[:, :])
                for t in range(NT):
                    ps_tr = psp.tile([128, 128], F32, tag="ps")
                    nc.tensor.transpose(
                        ps_tr[:, :], xct_sb[:, 128 * t:128 * (t + 1)],
                        identf[:, :],
                    )
                    nc.vector.scalar_tensor_tensor(
                        out=s_tiles[t][:, 128 * j:128 * (j + 1)],
                        in0=ps_tr[:, :],
                        scalar=-2.0,
                        in1=c2_sb[:, 128 * j:128 * (j + 1)],
                        op0=mybir.AluOpType.mult,
                        op1=mybir.AluOpType.add,
                    )

            kt_sb = constp.tile([NC, TOK], BF16, tag="kt_sb")
            for t in range(NT):
                score = s_tiles[t]
                score3 = score[:, :].rearrange("p (c k) -> p c k", k=K)
                m = workp.tile([128, NC], F32, tag="m")
                nc.vector.tensor_reduce(
                    out=m[:, :], in_=score3, axis=mybir.AxisListType.X,
                    op=mybir.AluOpType.min,
                )
                eq = workp.tile([128, CK], F32, tag="eq")
                nc.vector.tensor_tensor(
                    out=eq[:, :].rearrange("p (c k) -> p c k", k=K),
                    in0=score3,
                    in1=m[:, :].unsqueeze(2).broadcast_to([128, NC, K]),
                    op=mybir.AluOpType.is_equal,
                )
                cand = workp.tile([128, CK], F32, tag="cand")
                nc.vector.scalar_tensor_tensor(
                    out=cand[:, :], in0=eq[:, :], scalar=-BIG,
                    in1=iota_sb[:, :],
                    op0=mybir.AluOpType.mult, op1=mybir.AluOpType.add,
                )
                kmin = workp.tile([128, NC], F32, tag="kmin")
                nc.vector.tensor_reduce(
                    out=kmin[:, :],
                    in_=cand[:, :].rearrange("p (c k) -> p c k", k=K),
                    axis=mybir.AxisListType.X, op=mybir.AluOpType.min,
                )
                kminb = workp.tile([128, NC], BF16, tag="kminb")
                nc.vector.tensor_scalar_add(kminb[:, :], kmin[:, :], BIG)
                # transpose kmin [n, c] -> [c, n] (values 0..15, exact bf16)
                ps_kt = psp.tile([NC, 128], BF16, tag="ps")
                nc.tensor.transpose(ps_kt[:, :], kminb[:, :], identb[:, :])
                nc.vector.tensor_copy(
                    out=kt_sb[:, 128 * t:128 * (t + 1)], in_=ps_kt[:, :])

            # ---- phase 2: AllGather the indices (64KB/core) ----------------
            nc.sync.dma_start(out=kt_bounce[:, :], in_=kt_sb[:, :])
            nc.gpsimd.collective_compute(
                "AllGather",
                mybir.AluOpType.bypass,
                replica_groups=[list(range(N_CORES))],
                ins=[kt_bounce.opt()],
                outs=[kt_all.opt()],
            )

            # ---- phase 3: lut = centroids @ weight ------------------------
            # 3-pass bf16 hi/lo (cbdh*wh + cbdh*wl + cbdl*wh): ~2^-18 per
            # product, bf16 matmul speed, col-packs 4 pairs per PSUM bank.
            lut_sb = lutp.tile([128, NCHUNK * OSH], F32, tag="lut")
            for j in range(NCHUNK):
                ps_lut = psp.tile([128, OSH], F32, tag="ps")
                for mcol in range(4):
                    p = 4 * j + mcol
                    wh_t = wp.tile([128, OSH], BF16, tag="wht")
                    nc.sync.dma_start(
                        out=wh_t[:, :],
                        in_=wsh_h.ap()[128 * p:128 * (p + 1), :])
                    wl_t = wp.tile([128, OSH], BF16, tag="wlt")
                    nc.sync.dma_start(
                        out=wl_t[:, :],
                        in_=wsh_l.ap()[128 * p:128 * (p + 1), :])
                    passes = [(cbdh_sb, wh_t), (cbdh_sb, wl_t),
                              (cbdl_sb, wh_t)]
                    for i, (cb, wt) in enumerate(passes):
                        nc.tensor.matmul(
                            ps_lut[32 * mcol:32 * (mcol + 1), :],
                            lhsT=cb[:, 32 * p:32 * (p + 1)],
                            rhs=wt[:, :],
                            start=(i == 0), stop=(i == 2),
                            tile_position=(0, 32 * mcol),
                        )
                nc.vector.tensor_copy(
                    out=lut_sb[:, OSH * j:OSH * (j + 1)], in_=ps_lut[:, :])

            # ---- phase 4: global absmax -> AllReduce(max) -> scale ---------
            mx8 = constp.tile([128, NCHUNK], F32, tag="mx8")
            for j in range(NCHUNK):
                nc.vector.tensor_reduce(
                    out=mx8[:, j:j + 1],
                    in_=lut_sb[:, OSH * j:OSH * (j + 1)],
                    axis=mybir.AxisListType.X, op=mybir.AluOpType.max,
                    apply_absolute_value=True,
                )
            mxl = constp.tile([128, 1], F32, tag="mxl")
            nc.vector.tensor_reduce(
                out=mxl[:, :], in_=mx8[:, :], axis=mybir.AxisListType.X,
                op=mybir.AluOpType.max,
            )
            mxp = constp.tile([128, 1], F32, tag="mxp")
            nc.gpsimd.partition_all_reduce(
                mxp[:, :], mxl[:, :], channels=128,
                reduce_op=bass_isa.ReduceOp.max,
            )
            zrow = constp.tile([1, 16], F32, tag="zrow")
            nc.vector.memset(zrow[:, :], 0.0)
            nc.vector.tensor_copy(out=zrow[0:1, 0:1], in_=mxp[0:1, 0:1])
            nc.sync.dma_start(out=mx_in[:, :], in_=zrow[:, :])
            nc.gpsimd.collective_compute(
                "AllReduce",
                mybir.AluOpType.max,
                replica_groups=[list(range(N_CORES))],
                ins=[mx_in.opt()],
                outs=[mx_out.opt()],
            )
            gmax_row = constp.tile([1, 16], F32, tag="gmax_row")
            nc.sync.dma_start(out=gmax_row[:, :], in_=mx_out[:, :])
            gmax = constp.tile([128, 1], F32, tag="gmax")
            nc.gpsimd.partition_broadcast(gmax[:, :], gmax_row[0:1, 0:1])
            # s = gmax/127 and 1/s via reciprocal (DVE has no divide); the
            # <=2ulp drift is far inside the error budget.
            s_col = constp.tile([128, 1], F32, tag="s_col")
            nc.vector.tensor_scalar(
                out=s_col[:, :], in0=gmax[:, :],
                scalar1=float(np.float32(1.0) / np.float32(127.0)),
                scalar2=None, op0=mybir.AluOpType.mult,
            )
            rgmax = constp.tile([128, 1], F32, tag="rgmax")
            nc.vector.reciprocal(rgmax[:, :], gmax[:, :])
            inv_s = constp.tile([128, 1], F32, tag="inv_s")
            nc.vector.tensor_scalar(
                out=inv_s[:, :], in0=rgmax[:, :], scalar1=127.0, scalar2=None,
                op0=mybir.AluOpType.mult,
            )

            # ---- phase 5: quantize-dequantize lut -> bf16 ------------------
            lutdq = lutp.tile([128, NCHUNK * OSH], BF16, tag="lutdq")
            for j in range(NCHUNK):
                qm = wp.tile([128, OSH], F32, tag="qm")
                nc.vector.tensor_scalar(
                    out=qm[:, :], in0=lut_sb[:, OSH * j:OSH * (j + 1)],
                    scalar1=inv_s[:, 0:1], scalar2=MAGIC,
                    op0=mybir.AluOpType.mult, op1=mybir.AluOpType.add,
                )
                nc.vector.tensor_scalar(
                    out=lutdq[:, OSH * j:OSH * (j + 1)], in0=qm[:, :],
                    scalar1=MAGIC, scalar2=s_col[:, 0:1],
                    op0=mybir.AluOpType.subtract, op1=mybir.AluOpType.mult,
                )

            # ---- phase 6: expand one-hots + gather matmul ------------------
            for r in range(N_CORES):
                ktr = stgp.tile([NC, TOK], BF16, tag="ktr", bufs=4)
                nc.sync.dma_start(
                    out=ktr[:, :],
                    in_=kt_all[NC * r:NC * (r + 1), :])
                stg_tiles = []
                for j in range(NCHUNK):
                    # replicate kmin rows onto the 16 k-partitions of chunk j
                    ps_rep = psp.tile([128, TOK], F32, tag="ps")
                    nc.tensor.matmul(
                        ps_rep[:, :],
                        lhsT=emat_sb[:, 128 * j:128 * (j + 1)],
                        rhs=ktr[:, :],
                        start=True, stop=True,
                    )
                    stg_t = stgp.tile([128, TOK], BF16, tag="stg")
                    nc.vector.tensor_scalar(
                        out=stg_t[:, :], in0=ps_rep[:, :],
                        scalar1=kcol_sb[:, 0:1], scalar2=None,
                        op0=mybir.AluOpType.is_equal,
                    )
                    stg_tiles.append(stg_t)
                for t in range(NT):
                    ps_o = psp.tile([128, OSH], F32, tag="ps")
                    for j in range(NCHUNK):
                        nc.tensor.matmul(
                            ps_o[:, :],
                            lhsT=stg_tiles[j][:, 128 * t:128 * (t + 1)],
                            rhs=lutdq[:, OSH * j:OSH * (j + 1)],
                            start=(j == 0), stop=(j == NCHUNK - 1),
                        )
                    o_sb = outp.tile([128, OSH], BF16, tag="outsb")
                    nc.vector.tensor_tensor(
                        out=o_sb[:, :], in0=ps_o[:, :], in1=bias_sb[:, :],
                        op=mybir.AluOpType.add,
                    )
                    row0 = TOK * r + 128 * t
                    nc.sync.dma_start(
                        out=out.ap()[row0:row0 + 128, :], in_=o_sb[:, :])

    nc.compile()
    return nc


def _prep_inputs(x, centroids, weight, bias):
    import ml_dtypes

    x = np.ascontiguousarray(np.asarray(x, dtype=np.float32)).reshape(BN, IN_F)
    cent = np.asarray(centroids, dtype=np.float32)
    w = np.asarray(weight, dtype=np.float32)
    bias = np.asarray(bias, dtype=np.float32)

    c2 = (cent ** 2).sum(axis=-1).reshape(CK)  # [1024] flat (c,k)
    c2r = np.ascontiguousarray(np.broadcast_to(c2, (128, CK)))
    iota = np.tile(np.arange(K, dtype=np.float32), NC)
    iotar = np.ascontiguousarray(np.broadcast_to(iota, (128, CK)))
    cbd = np.zeros((128, CK), np.float32)
    for p in range(NPAIR):
        cbd[0:SUBV, 32 * p:32 * p + K] = cent[2 * p].T
        cbd[SUBV:128, 32 * p + K:32 * p + 2 * K] = cent[2 * p + 1].T
    cbd_h = cbd.astype(ml_dtypes.bfloat16)
    cbd_l = (cbd - cbd_h.astype(np.float32)).astype(ml_dtypes.bfloat16)
    # E[c, ck] = 1 where ck // 16 == c  (replication matrix)
    emat = (np.arange(CK)[None, :] // K == np.arange(NC)[:, None]).astype(
        ml_dtypes.bfloat16)
    kcol = np.ascontiguousarray(
        (np.arange(128, dtype=np.float32) % K).reshape(128, 1))

    in_maps = []
    for r in range(N_CORES):
        xT_r = np.ascontiguousarray(x[TOK * r:TOK * (r + 1)].T)
        w_r = np.ascontiguousarray(w[:, :, OSH * r:OSH * (r + 1)]).reshape(
            IN_F, OSH)
        w_h = w_r.astype(ml_dtypes.bfloat16)
        w_l = (w_r - w_h.astype(np.float32)).astype(ml_dtypes.bfloat16)
        bias_r = np.ascontiguousarray(
            np.broadcast_to(bias[OSH * r:OSH * (r + 1)], (128, OSH)))
        in_maps.append({
            "xT": xT_r, "cbd": cbd, "c2r": c2r, "iotar": iotar,
            "wsh_h": w_h, "wsh_l": w_l, "cbd_h": cbd_h, "cbd_l": cbd_l,
            "biasr": bias_r, "emat": emat, "kcol": kcol,
        })
    return in_maps


def kernel(x, centroids, weight, inverse_temperature_logit, bias,
           **_unused) -> np.ndarray:
    if "nc" not in _CACHE:
        _CACHE["nc"] = _build()
    nc = _CACHE["nc"]
    in_maps = _prep_inputs(x, centroids, weight, bias)
    res = run_bass_kernel_spmd(nc, in_maps, core_ids=list(range(N_CORES)))
    out = np.concatenate(
        [res.results[r]["out"].astype(np.float32) for r in range(N_CORES)],
        axis=1)
    return out.reshape(2, BN // 2, OUT_F)
